# revision 8
# baseline (speedup 1.0000x reference)
"""Trainium2 Bass kernel for nn_Decoder_17214228922493.

32-step LSTM decoder: B=64, H=1536, input=1024, applied to a constant input.
    xg = x @ W_ih.T + b_ih + b_hh                      (recomputed per step)
    per step: gates = xg + h @ W_hh.T ; LSTM cell update ; emit h

Sharding: tensor-parallel over the gate dimension (8 cores x 768 gate
columns); after every step the 8 h^T slices are re-assembled with an
AllGather (mesh, ~5.5us).

The per-step serial chain is h -> pack -> bounce DMA -> AllGather ->
reload -> h-matmuls.  Everything around it is tuned to that chain:

  - wire layout [128, 96] per slice: cols 0:64 a clean k-tile (dims
    0:128), cols 64:96 the last 64 dims batch-split.  Bounce writes and
    reloads are then large-segment/contiguous DMAs instead of the 128-byte
    strided patterns a [192, 64] slice layout forces (which cost 2.5-4.5us
    per DMA in receipt+RMW).
  - reload = two [128, 4*96] DMAs (sync + scalar), blocks 0-3 / 4-7 of the
    gathered [1024, 96]; four small SBUF->SBUF fixup DMAs rebuild the
    batch-split halves into 4 clean k-tiles while the 8 A-part matmuls run.
  - the x-matmul recompute starts right at the previous h-ready and ~34
    dummy matmuls follow it, keeping the PE busy through the AllGather
    window so HAM never rethrottles the critical h-matmuls to 1.2GHz.
  - the output writeback is pinned behind the reload so its HBM traffic
    stays off the AllGather span.

Matmul operands are bf16; PSUM accumulation and cell state stay fp32.
"""

import sys

if "/opt/trn_rl_repo" not in sys.path:
    sys.path.insert(0, "/opt/trn_rl_repo")

from contextlib import ExitStack

import ml_dtypes
import numpy as np

import concourse.bass as bass
import concourse.mybir as mybir
import concourse.tile as tile
from concourse import bacc
from concourse import bass_utils
from concourse._compat import get_trn_type

F32 = mybir.dt.float32
BF16 = mybir.dt.bfloat16
R = 8          # cores
B = 64         # batch
H = 1536       # hidden
HL = H // R    # 192 per-core hidden slice
IN = 1024      # lstm input size
KA = 1152      # augmented input contraction (1024 + bias row, padded to 9*128)
NG = 4 * HL    # 768 gate columns per core
S = 32         # steps
NH = 384       # first gate-group width [g|f]
KAT = KA // 128  # 9 k-tiles for the input matmul
NDUMMY = 34    # free-running 512-wide PE filler matmuls per step

_CACHE = {}


def _build():
    nc = bacc.Bacc(
        get_trn_type() or "TRN2",
        target_bir_lowering=False,
        debug=False,
        num_devices=R,
    )

    xT = nc.dram_tensor("xT", [KA, B], BF16, kind="ExternalInput")
    wih = nc.dram_tensor("wih", [KA, NG], BF16, kind="ExternalInput")
    whh = nc.dram_tensor("whh", [H, NG], BF16, kind="ExternalInput")
    g0pack = nc.dram_tensor("g0pack", [128, R * 96], BF16, kind="ExternalInput")
    c0 = nc.dram_tensor("c0", [B, HL], F32, kind="ExternalInput")
    iden = nc.dram_tensor("iden", [B, B], BF16, kind="ExternalInput")
    out = nc.dram_tensor("out", [S, 128, 96], BF16, kind="ExternalOutput")

    bounces = [
        nc.dram_tensor(f"bounce{t}", [128, 96], BF16, kind="Internal")
        for t in range(S)
    ]
    gaths = [
        nc.dram_tensor(
            f"gath{t}", [R * 128, 96], BF16, kind="Internal", addr_space="Shared"
        )
        for t in range(S - 1)
    ]

    sig = mybir.ActivationFunctionType.Sigmoid
    tanh = mybir.ActivationFunctionType.Tanh

    with ExitStack() as ctx:
        tc = ctx.enter_context(tile.TileContext(nc))
        wpool = ctx.enter_context(tc.tile_pool(name="w", bufs=1))
        cpool = ctx.enter_context(tc.tile_pool(name="cst", bufs=1))
        hpool = ctx.enter_context(tc.tile_pool(name="h", bufs=2))
        spool = ctx.enter_context(tc.tile_pool(name="s", bufs=3))
        gpool = ctx.enter_context(tc.tile_pool(name="g", bufs=1, space="PSUM"))
        tpool = ctx.enter_context(tc.tile_pool(name="t", bufs=1, space="PSUM"))
        dpool = ctx.enter_context(tc.tile_pool(name="d", bufs=1, space="PSUM"))

        whh_t = []
        for k in range(H // 128):
            w = wpool.tile([128, NG], BF16, tag=f"whh{k}", name=f"whh{k}")
            nc.sync.dma_start(w[:], whh[128 * k : 128 * (k + 1), :])
            whh_t.append(w)
        wih_t = []
        for k in range(KAT):
            w = wpool.tile([128, NG], BF16, tag=f"wih{k}", name=f"wih{k}")
            nc.sync.dma_start(w[:], wih[128 * k : 128 * (k + 1), :])
            wih_t.append(w)
        x_t = []
        for k in range(KAT):
            xx = wpool.tile([128, B], BF16, tag=f"x{k}", name=f"x{k}")
            nc.sync.dma_start(xx[:], xT[128 * k : 128 * (k + 1), :])
            x_t.append(xx)
        iden_t = cpool.tile([B, B], BF16, tag="iden")
        nc.sync.dma_start(iden_t[:], iden[:])

        def load_g(src):
            """Reload gathered [1024, 96] into [128, 8*96] in two DMAs."""
            g = hpool.tile([128, R * 96], BF16, tag="grx", name="grx")
            dmas = []
            for half, eng in ((0, nc.sync), (1, nc.scalar)):
                src_ap = src[512 * half : 512 * (half + 1), :].rearrange(
                    "(j p) c -> p j c", j=4
                )
                dst_ap = g[:, 384 * half : 384 * (half + 1)].rearrange(
                    "p (j c) -> p j c", j=4
                )
                dmas.append(eng.dma_start(dst_ap, src_ap))
            return g, dmas

        def fixup(g):
            """4 SBUF->SBUF DMAs: batch-split B-chunks -> 4 clean k-tiles."""
            bf = hpool.tile([128, 4 * B], BF16, tag="bfix", name="bfix")

            def gview(p0, q0):
                return g[p0 : p0 + 64].rearrange("p (u q) -> p u q", u=4)[
                    :, :, q0 : q0 + 32
                ]

            def bview(p0, q0):
                return bf[p0 : p0 + 64].rearrange("p (u q) -> p u q", u=4)[
                    :, :, q0 : q0 + 32
                ]

            for eng, src, dst in (
                (nc.sync, gview(0, 64), bview(0, 0)),
                (nc.sync, gview(64, 64), bview(0, 32)),
                (nc.scalar, gview(0, 160), bview(64, 0)),
                (nc.scalar, gview(64, 160), bview(64, 32)),
            ):
                eng.dma_start(dst, src)
            return bf

        g_rx = hpool.tile([128, R * 96], BF16, tag="grx", name="grx0")
        nc.sync.dma_start(g_rx[:], g0pack[:])
        b_fx = fixup(g_rx)
        reload_dmas = None

        c_t = spool.tile([B, HL], F32, tag="c")
        nc.sync.dma_start(c_t[:], c0[:])

        dummy_ps = dpool.tile([B, 512], F32, tag="dummy")

        for t in range(S):
            ps = []
            group_slices = [slice(0, NH), slice(NH, NH + HL), slice(NH + HL, NG)]
            for n, nsl in enumerate(group_slices):
                p = gpool.tile(
                    [B, nsl.stop - nsl.start], F32, tag=f"g{n}", name=f"ps{n}"
                )
                for k in range(KAT):
                    nc.tensor.matmul(
                        p[:], x_t[k][:], wih_t[k][:, nsl], start=(k == 0), stop=False
                    )
                ps.append(p)
            if t > 0:
                # free-running filler: keeps HAM at 2.4GHz through the
                # AllGather window so the critical h-matmuls run warm
                for _ in range(NDUMMY):
                    nc.tensor.matmul(
                        dummy_ps[:], x_t[0][:], whh_t[0][:, 0:512],
                        start=True, stop=True,
                    )
            for n, nsl in enumerate(group_slices):
                for j in range(R):
                    nc.tensor.matmul(
                        ps[n][:],
                        g_rx[:, 96 * j : 96 * j + 64],
                        whh_t[j][:, nsl],
                        start=False,
                        stop=False,
                    )
                for u in range(4):
                    nc.tensor.matmul(
                        ps[n][:],
                        b_fx[:, B * u : B * (u + 1)],
                        whh_t[8 + u][:, nsl],
                        start=False,
                        stop=(u == 3),
                    )

            # eltwise: c-chain overlaps the later gate groups' matmuls
            tg = spool.tile([B, HL], F32, tag="tg")
            nc.scalar.activation(tg[:], ps[0][:, 0:HL], tanh)
            s_f = spool.tile([B, HL], F32, tag="sf")
            nc.scalar.activation(s_f[:], ps[0][:, HL : 2 * HL], sig)
            m1 = spool.tile([B, HL], F32, tag="m1")
            nc.vector.tensor_mul(m1[:], s_f[:], c_t[:])  # f*c
            s_i = spool.tile([B, HL], F32, tag="si")
            nc.scalar.activation(s_i[:], ps[1][:], sig)
            m2 = spool.tile([B, HL], F32, tag="m2")
            nc.vector.tensor_mul(m2[:], s_i[:], tg[:])  # i*g
            c_new = spool.tile([B, HL], F32, tag="c")
            nc.vector.tensor_add(c_new[:], m1[:], m2[:])
            c_t = c_new
            tc_sb = spool.tile([B, HL], BF16, tag="tc")
            nc.scalar.activation(tc_sb[:], c_new[:], tanh)
            s_o = spool.tile([B, HL], BF16, tag="so")
            nc.scalar.activation(s_o[:], ps[2][:], sig)
            h_sb = spool.tile([B, HL], BF16, tag="hsb")
            nc.vector.tensor_mul(h_sb[:], s_o[:], tc_sb[:])

            # pack h^T slice into the [128, 96] wire layout on the PE
            pk_ps = tpool.tile([128, 96], BF16, tag="pkps")
            nc.tensor.transpose(pk_ps[:, 0:64], h_sb[:, 0:128], iden_t[:, 0:64])
            nc.tensor.transpose(pk_ps[:, 64:96], h_sb[:, 64:HL], iden_t[:, 32:64])
            nc.tensor.transpose(
                pk_ps[0:64, 64:96], h_sb[:, 128:HL], iden_t[:, 0:32]
            )
            pk_sb = spool.tile([128, 96], BF16, tag="pksb")
            nc.vector.tensor_copy(pk_sb[:], pk_ps[:])
            nc.sync.dma_start(bounces[t][:], pk_sb[:])

            if t < S - 1:
                nc.gpsimd.collective_compute(
                    "AllGather",
                    mybir.AluOpType.bypass,
                    replica_groups=[list(range(R))],
                    ins=[bounces[t][:]],
                    outs=[gaths[t][:]],
                )
                g_rx, reload_dmas = load_g(gaths[t])
                b_fx = fixup(g_rx)

            # output writeback pinned behind the reload, off the AG span
            od = nc.scalar.dma_start(out[t, :, :], pk_sb[:])
            if reload_dmas is not None:
                tile.add_dep_helper(
                    od.ins, reload_dmas[1].ins, sync=True,
                    reason="keep output writeback off the AllGather span",
                )

    nc.compile()
    return nc


def _pack192(M):
    """[192, 64] slice -> [128, 96] wire layout (A | batch-split B)."""
    P = np.empty((128, 96), M.dtype)
    P[:, 0:64] = M[0:128]
    P[0:64, 64:96] = M[128:192, 0:32]
    P[64:128, 64:96] = M[128:192, 32:64]
    return P


def _prep_inputs(sequence, hidden_state, cell_state, W_ih, W_hh, b_ih, b_hh):
    x = np.asarray(sequence, np.float32)[0]          # [64, 1024]
    h0 = np.asarray(hidden_state, np.float32)[0]     # [64, 1536]
    c0f = np.asarray(cell_state, np.float32)[0]
    W_ih = np.asarray(W_ih, np.float32)
    W_hh = np.asarray(W_hh, np.float32)
    b = (np.asarray(b_ih, np.float32) + np.asarray(b_hh, np.float32))

    bf = ml_dtypes.bfloat16
    xT = np.zeros((KA, B), np.float32)
    xT[:IN] = x.T
    xT[IN] = 1.0
    xT = xT.astype(bf)
    h0T = h0.T.astype(np.float32)                    # [1536, 64]
    iden = np.eye(B, dtype=bf)

    in_maps = []
    for r in range(R):
        sl = np.arange(r * HL, (r + 1) * HL)
        # per-core gate column order: g, f, i, o
        sel = np.concatenate([2 * H + sl, H + sl, sl, 3 * H + sl])
        wa = np.zeros((KA, NG), np.float32)
        wa[:IN] = W_ih[sel].T
        wa[IN] = b[sel]

        # contraction row order: 8 A k-tiles (rank-major), 4 B pair k-tiles
        rows = []
        for j in range(R):
            rows.append(np.arange(HL * j, HL * j + 128))
        for u in range(4):
            rows.append(np.arange(HL * 2 * u + 128, HL * 2 * u + HL))
            rows.append(np.arange(HL * (2 * u + 1) + 128, HL * (2 * u + 1) + HL))
        rows = np.concatenate(rows)
        whh_r = np.ascontiguousarray(W_hh[sel].T[rows]).astype(bf)

        g0 = np.empty((128, R * 96), np.float32)
        for j in range(R):
            g0[:, 96 * j : 96 * (j + 1)] = _pack192(h0T[HL * j : HL * (j + 1)])

        in_maps.append(
            {
                "xT": xT,
                "wih": wa.astype(bf),
                "whh": whh_r,
                "g0pack": g0.astype(bf),
                "c0": np.ascontiguousarray(c0f[:, sl]),
                "iden": iden,
            }
        )
    return in_maps


def kernel(**inputs) -> np.ndarray:
    if "nc" not in _CACHE:
        _CACHE["nc"] = _build()
    nc = _CACHE["nc"]
    in_maps = _prep_inputs(**inputs)
    res = bass_utils.run_bass_kernel_spmd(nc, in_maps, core_ids=list(range(R)))
    preds = np.empty((S, B, H), np.float32)
    for r in range(R):
        o = np.asarray(res.results[r]["out"], np.float32)  # [32, 128, 96]
        sl = slice(r * HL, r * HL + 128)
        preds[:, :, sl] = np.transpose(o[:, :, 0:64], (0, 2, 1))
        tail = slice(r * HL + 128, (r + 1) * HL)
        preds[:, 0:32, tail] = np.transpose(o[:, 0:64, 64:96], (0, 2, 1))
        preds[:, 32:64, tail] = np.transpose(o[:, 64:128, 64:96], (0, 2, 1))
    return preds


# revision 16
# speedup vs baseline: 1.1808x; 1.1808x over previous
"""Trainium2 Bass kernel for nn_Decoder_17214228922493.

32-step LSTM decoder: B=64, H=1536, input=1024, applied to a constant input.
    xg = x @ W_ih.T + b_ih + b_hh                      (recomputed per step)
    per step: gates = xg + h @ W_hh.T ; LSTM cell update ; emit h

Sharding: tensor-parallel over the gate dimension (8 cores x 768 gate
columns); after every step the 8 h^T slices are re-assembled with an
AllGather (mesh, ~5.5us).

The per-step serial chain is h -> pack -> bounce DMA -> AllGather ->
reload -> h-matmuls.  Everything around it is tuned to that chain:

  - wire layout [128, 96] per slice: cols 0:64 a clean k-tile (dims
    0:128), cols 64:96 the last 64 dims batch-split.  Bounce writes and
    reloads are then large-segment/contiguous DMAs instead of the 128-byte
    strided patterns a [192, 64] slice layout forces (which cost 2.5-4.5us
    per DMA in receipt+RMW).
  - reload = two [128, 4*96] DMAs (sync + scalar), blocks 0-3 / 4-7 of the
    gathered [1024, 96]; four small SBUF->SBUF fixup DMAs rebuild the
    batch-split halves into 4 clean k-tiles while the 8 A-part matmuls run.
  - the x-matmul recompute starts right at the previous h-ready and ~34
    dummy matmuls follow it, keeping the PE busy through the AllGather
    window so HAM never rethrottles the critical h-matmuls to 1.2GHz.
  - the output writeback is pinned behind the reload so its HBM traffic
    stays off the AllGather span.

Matmul operands are bf16; PSUM accumulation and cell state stay fp32.
"""

import sys

if "/opt/trn_rl_repo" not in sys.path:
    sys.path.insert(0, "/opt/trn_rl_repo")

from contextlib import ExitStack

import ml_dtypes
import numpy as np

import concourse.bass as bass
import concourse.mybir as mybir
import concourse.tile as tile
from concourse import bacc
from concourse import bass_utils
from concourse._compat import get_trn_type

F32 = mybir.dt.float32
BF16 = mybir.dt.bfloat16
R = 8          # cores
B = 64         # batch
H = 1536       # hidden
HL = H // R    # 192 per-core hidden slice
IN = 1024      # lstm input size
KA = 1152      # augmented input contraction (1024 + bias row, padded to 9*128)
NG = 4 * HL    # 768 gate columns per core
S = 32         # steps
NH = 384       # first gate-group width [g|f]
KAT = KA // 128  # 9 k-tiles for the input matmul
NDUMMY = 27    # 512-wide PE filler matmuls per step, anchored to the bounce

_CACHE = {}


def _build():
    nc = bacc.Bacc(
        get_trn_type() or "TRN2",
        target_bir_lowering=False,
        debug=False,
        num_devices=R,
    )

    xT = nc.dram_tensor("xT", [KA, B], BF16, kind="ExternalInput")
    wih = nc.dram_tensor("wih", [KA, NG], BF16, kind="ExternalInput")
    whh = nc.dram_tensor("whh", [H, NG], BF16, kind="ExternalInput")
    g0pack = nc.dram_tensor("g0pack", [128, R * 96], BF16, kind="ExternalInput")
    b0fix = nc.dram_tensor("b0fix", [128, 4 * B], BF16, kind="ExternalInput")
    c0 = nc.dram_tensor("c0", [B, HL], F32, kind="ExternalInput")
    iden = nc.dram_tensor("iden", [B, B], BF16, kind="ExternalInput")
    out = nc.dram_tensor("out", [S, 128, 96], BF16, kind="ExternalOutput")

    bounces = [
        nc.dram_tensor(f"bounce{t}", [128, 96], BF16, kind="Internal")
        for t in range(S)
    ]
    gaths = [
        nc.dram_tensor(
            f"gath{t}", [R * 128, 96], BF16, kind="Internal", addr_space="Shared"
        )
        for t in range(S - 1)
    ]

    sig = mybir.ActivationFunctionType.Sigmoid
    tanh = mybir.ActivationFunctionType.Tanh

    with ExitStack() as ctx:
        tc = ctx.enter_context(tile.TileContext(nc))
        wpool = ctx.enter_context(tc.tile_pool(name="w", bufs=1))
        cpool = ctx.enter_context(tc.tile_pool(name="cst", bufs=1))
        hpool = ctx.enter_context(tc.tile_pool(name="h", bufs=2))
        spool = ctx.enter_context(tc.tile_pool(name="s", bufs=3))
        gpool = ctx.enter_context(tc.tile_pool(name="g", bufs=1, space="PSUM"))
        tpool = ctx.enter_context(tc.tile_pool(name="t", bufs=1, space="PSUM"))
        dpool = ctx.enter_context(tc.tile_pool(name="d", bufs=1, space="PSUM"))

        whh_t = []
        for k in range(H // 128):
            w = wpool.tile([128, NG], BF16, tag=f"whh{k}", name=f"whh{k}")
            nc.sync.dma_start(w[:], whh[128 * k : 128 * (k + 1), :])
            whh_t.append(w)
        wih_t = []
        for k in range(KAT):
            w = wpool.tile([128, NG], BF16, tag=f"wih{k}", name=f"wih{k}")
            nc.sync.dma_start(w[:], wih[128 * k : 128 * (k + 1), :])
            wih_t.append(w)
        x_t = []
        for k in range(KAT):
            xx = wpool.tile([128, B], BF16, tag=f"x{k}", name=f"x{k}")
            nc.sync.dma_start(xx[:], xT[128 * k : 128 * (k + 1), :])
            x_t.append(xx)
        iden_t = cpool.tile([B, B], BF16, tag="iden")
        nc.sync.dma_start(iden_t[:], iden[:])

        def load_g(src):
            """Reload gathered [1024, 96] into [128, 8*96] in two DMAs."""
            g = hpool.tile([128, R * 96], BF16, tag="grx", name="grx")
            dmas = []
            for half, eng in ((0, nc.sync), (1, nc.scalar)):
                src_ap = src[512 * half : 512 * (half + 1), :].rearrange(
                    "(j p) c -> p j c", j=4
                )
                dst_ap = g[:, 384 * half : 384 * (half + 1)].rearrange(
                    "p (j c) -> p j c", j=4
                )
                dmas.append(eng.dma_start(dst_ap, src_ap))
            return g, dmas

        def fixup(src_dram):
            """4 DRAM->SBUF DMAs straight off the gathered buffer: rebuild
            the batch-split B-chunks into 4 clean k-tiles, in parallel with
            the main reload (both gate only on the AllGather)."""
            bf = hpool.tile([128, 4 * B], BF16, tag="bfix", name="bfix")
            # blocks as (u, two): block 2u+tw at rows/cols (u, tw)
            g_r = src_dram.rearrange("(u tw p) c -> p u tw c", u=4, tw=2)

            def bview(p0, q0):
                return bf[p0 : p0 + 64].rearrange("p (u q) -> p u q", u=4)[
                    :, :, q0 : q0 + 32
                ]

            for eng, src, dst in (
                (nc.sync, g_r[0:64, :, 0, 64:96], bview(0, 0)),
                (nc.sync, g_r[64:128, :, 0, 64:96], bview(0, 32)),
                (nc.scalar, g_r[0:64, :, 1, 64:96], bview(64, 0)),
                (nc.scalar, g_r[64:128, :, 1, 64:96], bview(64, 32)),
            ):
                eng.dma_start(dst, src)
            return bf

        g_rx = hpool.tile([128, R * 96], BF16, tag="grx", name="grx0")
        nc.sync.dma_start(g_rx[:], g0pack[:])
        b_fx = hpool.tile([128, 4 * B], BF16, tag="bfix", name="bfix0")
        nc.scalar.dma_start(b_fx[:], b0fix[:])
        reload_dmas = None
        prev_bounce = None

        c_t = spool.tile([B, HL], F32, tag="c")
        nc.sync.dma_start(c_t[:], c0[:])

        dummy_ps = dpool.tile([B, 512], F32, tag="dummy")

        for t in range(S):
            if t > 0:
                # filler anchored to the previous bounce receipt: streams
                # through the AllGather+reload window so HAM stays at
                # 2.4GHz and the critical h-matmuls below enter warm
                for i in range(NDUMMY):
                    dmm = nc.tensor.matmul(
                        dummy_ps[:], x_t[0][:], whh_t[0][:, 0:512],
                        start=True, stop=True,
                    )
                    if i == 0:
                        tile.add_dep_helper(
                            dmm.ins, prev_bounce.ins, sync=True,
                            reason="filler fills the AllGather window",
                        )
                    last_dummy = dmm
            ps = []
            group_slices = [slice(0, NH), slice(NH, NH + HL), slice(NH + HL, NG)]
            for n, nsl in enumerate(group_slices):
                p = gpool.tile(
                    [B, nsl.stop - nsl.start], F32, tag=f"g{n}", name=f"ps{n}"
                )
                for k in range(KAT):
                    mm = nc.tensor.matmul(
                        p[:], x_t[k][:], wih_t[k][:, nsl], start=(k == 0), stop=False
                    )
                    if t > 0 and n == 0 and k == 0:
                        tile.add_dep_helper(
                            mm.ins, last_dummy.ins, sync=True,
                            reason="x-recompute streams after the filler",
                        )
                ps.append(p)
            for n, nsl in enumerate(group_slices):
                for j in range(R):
                    nc.tensor.matmul(
                        ps[n][:],
                        g_rx[:, 96 * j : 96 * j + 64],
                        whh_t[j][:, nsl],
                        start=False,
                        stop=False,
                    )
                for u in range(4):
                    nc.tensor.matmul(
                        ps[n][:],
                        b_fx[:, B * u : B * (u + 1)],
                        whh_t[8 + u][:, nsl],
                        start=False,
                        stop=(u == 3),
                    )

            # eltwise: c-chain overlaps the later gate groups' matmuls
            tg = spool.tile([B, HL], F32, tag="tg")
            nc.scalar.activation(tg[:], ps[0][:, 0:HL], tanh)
            s_f = spool.tile([B, HL], F32, tag="sf")
            nc.scalar.activation(s_f[:], ps[0][:, HL : 2 * HL], sig)
            m1 = spool.tile([B, HL], F32, tag="m1")
            nc.vector.tensor_mul(m1[:], s_f[:], c_t[:])  # f*c
            s_i = spool.tile([B, HL], F32, tag="si")
            nc.scalar.activation(s_i[:], ps[1][:], sig)
            m2 = spool.tile([B, HL], F32, tag="m2")
            nc.vector.tensor_mul(m2[:], s_i[:], tg[:])  # i*g
            c_new = spool.tile([B, HL], F32, tag="c")
            nc.vector.tensor_add(c_new[:], m1[:], m2[:])
            c_t = c_new
            tc_sb = spool.tile([B, HL], BF16, tag="tc")
            nc.scalar.activation(tc_sb[:], c_new[:], tanh)
            s_o = spool.tile([B, HL], BF16, tag="so")
            nc.scalar.activation(s_o[:], ps[2][:], sig)
            h_sb = spool.tile([B, HL], BF16, tag="hsb")
            nc.vector.tensor_mul(h_sb[:], s_o[:], tc_sb[:])

            # pack h^T slice into the [128, 96] wire layout on the PE
            pk_ps = tpool.tile([128, 96], BF16, tag="pkps")
            nc.tensor.transpose(pk_ps[:, 0:64], h_sb[:, 0:128], iden_t[:, 0:64])
            nc.tensor.transpose(pk_ps[:, 64:96], h_sb[:, 64:HL], iden_t[:, 32:64])
            nc.tensor.transpose(
                pk_ps[0:64, 64:96], h_sb[:, 128:HL], iden_t[:, 0:32]
            )
            pk_sb = spool.tile([128, 96], BF16, tag="pksb")
            nc.vector.tensor_copy(pk_sb[:], pk_ps[:])
            prev_bounce = nc.sync.dma_start(bounces[t][:], pk_sb[:])

            if t < S - 1:
                nc.gpsimd.collective_compute(
                    "AllGather",
                    mybir.AluOpType.bypass,
                    replica_groups=[list(range(R))],
                    ins=[bounces[t][:]],
                    outs=[gaths[t][:]],
                )
                g_rx, reload_dmas = load_g(gaths[t])
                b_fx = fixup(gaths[t])

            # output writeback pinned behind the reload, off the AG span
            od = nc.scalar.dma_start(out[t, :, :], pk_sb[:])
            if reload_dmas is not None:
                tile.add_dep_helper(
                    od.ins, reload_dmas[1].ins, sync=True,
                    reason="keep output writeback off the AllGather span",
                )

    nc.compile()
    return nc


def _pack192(M):
    """[192, 64] slice -> [128, 96] wire layout (A | batch-split B)."""
    P = np.empty((128, 96), M.dtype)
    P[:, 0:64] = M[0:128]
    P[0:64, 64:96] = M[128:192, 0:32]
    P[64:128, 64:96] = M[128:192, 32:64]
    return P


def _prep_inputs(sequence, hidden_state, cell_state, W_ih, W_hh, b_ih, b_hh):
    x = np.asarray(sequence, np.float32)[0]          # [64, 1024]
    h0 = np.asarray(hidden_state, np.float32)[0]     # [64, 1536]
    c0f = np.asarray(cell_state, np.float32)[0]
    W_ih = np.asarray(W_ih, np.float32)
    W_hh = np.asarray(W_hh, np.float32)
    b = (np.asarray(b_ih, np.float32) + np.asarray(b_hh, np.float32))

    bf = ml_dtypes.bfloat16
    xT = np.zeros((KA, B), np.float32)
    xT[:IN] = x.T
    xT[IN] = 1.0
    xT = xT.astype(bf)
    h0T = h0.T.astype(np.float32)                    # [1536, 64]
    iden = np.eye(B, dtype=bf)

    in_maps = []
    for r in range(R):
        sl = np.arange(r * HL, (r + 1) * HL)
        # per-core gate column order: g, f, i, o
        sel = np.concatenate([2 * H + sl, H + sl, sl, 3 * H + sl])
        wa = np.zeros((KA, NG), np.float32)
        wa[:IN] = W_ih[sel].T
        wa[IN] = b[sel]

        # contraction row order: 8 A k-tiles (rank-major), 4 B pair k-tiles
        rows = []
        for j in range(R):
            rows.append(np.arange(HL * j, HL * j + 128))
        for u in range(4):
            rows.append(np.arange(HL * 2 * u + 128, HL * 2 * u + HL))
            rows.append(np.arange(HL * (2 * u + 1) + 128, HL * (2 * u + 1) + HL))
        rows = np.concatenate(rows)
        whh_r = np.ascontiguousarray(W_hh[sel].T[rows]).astype(bf)

        g0 = np.empty((128, R * 96), np.float32)
        for j in range(R):
            g0[:, 96 * j : 96 * (j + 1)] = _pack192(h0T[HL * j : HL * (j + 1)])
        # t=0 fixup content: pair k-tile u = [dims 128:192 of rank 2u (rows
        # 0:64) ; dims 128:192 of rank 2u+1 (rows 64:128)], cols = batch
        b0 = np.empty((128, 4 * B), np.float32)
        for u in range(4):
            b0[0:64, B * u : B * (u + 1)] = h0T[HL * 2 * u + 128 : HL * 2 * u + HL]
            b0[64:128, B * u : B * (u + 1)] = h0T[
                HL * (2 * u + 1) + 128 : HL * (2 * u + 1) + HL
            ]

        in_maps.append(
            {
                "xT": xT,
                "wih": wa.astype(bf),
                "whh": whh_r,
                "g0pack": g0.astype(bf),
                "b0fix": b0.astype(bf),
                "c0": np.ascontiguousarray(c0f[:, sl]),
                "iden": iden,
            }
        )
    return in_maps


def kernel(**inputs) -> np.ndarray:
    if "nc" not in _CACHE:
        _CACHE["nc"] = _build()
    nc = _CACHE["nc"]
    in_maps = _prep_inputs(**inputs)
    res = bass_utils.run_bass_kernel_spmd(nc, in_maps, core_ids=list(range(R)))
    preds = np.empty((S, B, H), np.float32)
    for r in range(R):
        o = np.asarray(res.results[r]["out"], np.float32)  # [32, 128, 96]
        sl = slice(r * HL, r * HL + 128)
        preds[:, :, sl] = np.transpose(o[:, :, 0:64], (0, 2, 1))
        tail = slice(r * HL + 128, (r + 1) * HL)
        preds[:, 0:32, tail] = np.transpose(o[:, 0:64, 64:96], (0, 2, 1))
        preds[:, 32:64, tail] = np.transpose(o[:, 64:128, 64:96], (0, 2, 1))
    return preds


# revision 19
# speedup vs baseline: 1.2527x; 1.0609x over previous
"""Trainium2 Bass kernel for nn_Decoder_17214228922493.

32-step LSTM decoder: B=64, H=1536, input=1024, applied to a constant input.
    xg = x @ W_ih.T + b_ih + b_hh                      (once per step, see below)
    per step: gates = xg + h @ W_hh.T ; LSTM cell update ; emit h

Sharding: tensor-parallel over the gate dimension (8 cores x 768 gate
columns); after every step the 8 h^T slices are re-assembled with an
AllGather (mesh, ~6us).  Gate columns are reordered per core to
[g | f | i | o] so the c-chain eltwise overlaps the later gate groups'
matmuls.

The xg contribution is re-computed from x every step instead of being
injected from a saved tile: those matmuls have no dependency on h and
are explicitly held (add_dep_helper on the bounce DMA) so they execute
inside the AllGather window — free PE work off the critical path.

The final output writeback is pinned behind the h^T reload so its
HBM traffic never collides with the AllGather's SDMA work (unpinned it
alternates the AllGather between ~4.6us and ~7.4us).

Matmul operands are bf16 (fp32 matmul costs 2 PE passes); PSUM
accumulation and the cell-state arithmetic stay fp32.
"""

import sys

if "/opt/trn_rl_repo" not in sys.path:
    sys.path.insert(0, "/opt/trn_rl_repo")

from contextlib import ExitStack

import ml_dtypes
import numpy as np

import concourse.bass as bass
import concourse.mybir as mybir
import concourse.tile as tile
from concourse import bacc
from concourse import bass_utils
from concourse._compat import get_trn_type

F32 = mybir.dt.float32
BF16 = mybir.dt.bfloat16
R = 8          # cores
B = 64         # batch
H = 1536       # hidden
HL = H // R    # 192 per-core hidden slice
IN = 1024      # lstm input size
KA = 1152      # augmented input contraction (1024 + bias row, padded to 9*128)
NG = 4 * HL    # 768 gate columns per core
S = 32         # steps
NH = 384       # matmul moving free-dim (two groups of 384 = NG)
KHT = H // 128   # 12 k-tiles for the recurrent matmul
KAT = KA // 128  # 9 k-tiles for the input matmul

_CACHE = {}


def _build():
    nc = bacc.Bacc(
        get_trn_type() or "TRN2",
        target_bir_lowering=False,
        debug=False,
        num_devices=R,
    )

    xT = nc.dram_tensor("xT", [KA, B], BF16, kind="ExternalInput")
    wih = nc.dram_tensor("wih", [KA, NG], BF16, kind="ExternalInput")
    whh = nc.dram_tensor("whh", [H, NG], BF16, kind="ExternalInput")
    h0T = nc.dram_tensor("h0T", [H, B], BF16, kind="ExternalInput")
    c0 = nc.dram_tensor("c0", [B, HL], F32, kind="ExternalInput")
    iden = nc.dram_tensor("iden", [B, B], BF16, kind="ExternalInput")
    out = nc.dram_tensor("out", [S, HL, B], BF16, kind="ExternalOutput")

    bounces = [
        nc.dram_tensor(f"bounce{t}", [HL, B], BF16, kind="Internal") for t in range(S)
    ]
    gaths = [
        nc.dram_tensor(f"gath{t}", [H, B], BF16, kind="Internal", addr_space="Shared")
        for t in range(S - 1)
    ]

    sig = mybir.ActivationFunctionType.Sigmoid
    tanh = mybir.ActivationFunctionType.Tanh

    with ExitStack() as ctx:
        tc = ctx.enter_context(tile.TileContext(nc))
        wpool = ctx.enter_context(tc.tile_pool(name="w", bufs=1))
        cpool = ctx.enter_context(tc.tile_pool(name="cst", bufs=1))
        hpool = ctx.enter_context(tc.tile_pool(name="h", bufs=2))
        spool = ctx.enter_context(tc.tile_pool(name="s", bufs=3))
        gpool = ctx.enter_context(tc.tile_pool(name="g", bufs=2, space="PSUM"))
        tpool = ctx.enter_context(tc.tile_pool(name="t", bufs=1, space="PSUM"))

        whh_t = []
        for k in range(KHT):
            w = wpool.tile([128, NG], BF16, tag=f"whh{k}")
            nc.sync.dma_start(w[:], whh[128 * k : 128 * (k + 1), :])
            whh_t.append(w)
        wih_t = []
        for k in range(KAT):
            w = wpool.tile([128, NG], BF16, tag=f"wih{k}")
            nc.sync.dma_start(w[:], wih[128 * k : 128 * (k + 1), :])
            wih_t.append(w)
        x_t = []
        for k in range(KAT):
            xx = wpool.tile([128, B], BF16, tag=f"x{k}")
            nc.sync.dma_start(xx[:], xT[128 * k : 128 * (k + 1), :])
            x_t.append(xx)
        iden_t = cpool.tile([B, B], BF16, tag="iden")
        nc.sync.dma_start(iden_t[:], iden[:])

        # h^T lives in three tiles [128, 4*B] (k-tiles 0-3 | 4-7 | 8-11) so the
        # post-AllGather reload is three chunked DMAs and the first matmuls can
        # start as soon as the first chunk lands.
        # reload chunk plan: a tiny first chunk (2 k-tiles) lands with the
        # smallest DMA+receipt latency so the first matmuls start early; the
        # rest arrives on a parallel ring while they run.
        chunk_plan = [(0, 2, 0), (2, 5, 1), (7, 5, 0)]  # (k0, nk, engine)
        reload_engines = [nc.sync, nc.scalar]

        def load_hT(src):
            chunks = {}
            dmas = []
            for k0, nk, eng in chunk_plan:
                hc = hpool.tile([128, nk * B], BF16, tag=f"hh{k0}")
                src_ap = src.rearrange("(k p) n -> p k n", p=128)[
                    :, k0 : k0 + nk, :
                ]
                dmas.append(reload_engines[eng].dma_start(hc[:], src_ap))
                for k in range(k0, k0 + nk):
                    chunks[k] = (hc, k - k0)
            return chunks, dmas

        def h_tile(chunks, k):
            hc, off = chunks[k]
            return hc[:, B * off : B * (off + 1)]

        h_halves, _ = load_hT(h0T)
        c_t = spool.tile([B, HL], F32, tag="c")
        nc.sync.dma_start(c_t[:], c0[:])

        prev_bounce_dma = None
        delay_gate = None  # dict of taps once the first AllGather exists
        reload_pin = None  # reload DMA the output writeback must trail
        for t in range(S):
            # gates = x^T.T@wih + h^T.T@whh in three PSUM groups:
            # g0 = [g|f], g1 = [i], g2 = [o].  The 9 x-matmuls per group have
            # no h dependency and fill the preceding AllGather window.
            ps = []
            group_slices = [slice(0, NH), slice(NH, NH + HL), slice(NH + HL, NG)]
            for n, nsl in enumerate(group_slices):
                p = gpool.tile([B, nsl.stop - nsl.start], F32, tag=f"g{n}")
                for k in range(KAT):
                    mm = nc.tensor.matmul(
                        p[:], x_t[k][:], wih_t[k][:, nsl], start=(k == 0), stop=False
                    )
                    if k == 0 and delay_gate is not None:
                        # one DENSE x-matmul burst gated at ~AG-tail: the
                        # HAM clock-gate flips to 2.4GHz after ~3.4us of
                        # sustained work, so the burst ends warm right as the
                        # reload lands and the h-matmuls enter at full clock
                        tile.add_dep_helper(
                            mm.ins,
                            delay_gate["late"].ins,
                            sync=True,
                            reason="dense x-matmul burst at the AllGather tail",
                        )
                ps.append(p)
            for n, nsl in enumerate(group_slices):
                for k in range(KHT):
                    nc.tensor.matmul(
                        ps[n][:],
                        h_tile(h_halves, k),
                        whh_t[k][:, nsl],
                        start=False,
                        stop=(k == KHT - 1),
                    )

            # eltwise: group 0 = [g|f] finishes first -> tanh(g), sigmoid(f)
            # and f*c all run while group 1's h-matmuls still stream.
            tg = spool.tile([B, HL], F32, tag="tg")
            nc.scalar.activation(tg[:], ps[0][:, 0:HL], tanh)
            s_f = spool.tile([B, HL], F32, tag="sf")
            nc.scalar.activation(s_f[:], ps[0][:, HL : 2 * HL], sig)
            m1 = spool.tile([B, HL], F32, tag="m1")
            nc.vector.tensor_mul(m1[:], s_f[:], c_t[:])  # f*c
            # s_i fires after the [i] group while the [o] group still
            # streams, so the whole c-chain overlaps those matmuls and the
            # post-matmul tail is just sigmoid(o)->transpose->mul->DMA
            s_i = spool.tile([B, HL], F32, tag="si")
            nc.scalar.activation(s_i[:], ps[1][:], sig)
            m2 = spool.tile([B, HL], F32, tag="m2")
            nc.vector.tensor_mul(m2[:], s_i[:], tg[:])  # i*g
            c_new = spool.tile([B, HL], F32, tag="c")
            nc.vector.tensor_add(c_new[:], m1[:], m2[:])
            c_t = c_new
            tc_sb = spool.tile([B, HL], BF16, tag="tc")
            nc.scalar.activation(tc_sb[:], c_new[:], tanh)
            tp_tc = tpool.tile([96, 2 * B], BF16, tag="htc_ps")
            nc.tensor.transpose(tp_tc[:, 0:B], tc_sb[:, 0:96], iden_t[:])
            nc.tensor.transpose(tp_tc[:, B : 2 * B], tc_sb[:, 96:HL], iden_t[:])
            s_o = spool.tile([B, HL], BF16, tag="so")
            nc.scalar.activation(s_o[:], ps[2][:], sig)
            tp_so = tpool.tile([96, 2 * B], BF16, tag="hso")
            nc.tensor.transpose(tp_so[:, 0:B], s_o[:, 0:96], iden_t[:])
            nc.tensor.transpose(tp_so[:, B : 2 * B], s_o[:, 96:HL], iden_t[:])
            so_T = spool.tile([96, 2 * B], BF16, tag="soT")
            nc.vector.tensor_copy(so_T[:], tp_so[:])

            # h^T = s_o^T * tanh(c)^T straight into the bounce-layout tile
            htc = spool.tile([96, 2 * B], BF16, tag="htc")
            nc.vector.tensor_mul(htc[:], so_T[:], tp_tc[:])
            dst = bounces[t].rearrange("(b p) n -> p b n", p=96)
            src = htc.rearrange("p (b n) -> p b n", b=2)
            prev_bounce_dma = nc.sync.dma_start(dst, src)

            if t < S - 1:
                nc.gpsimd.collective_compute(
                    "AllGather",
                    mybir.AluOpType.bypass,
                    replica_groups=[list(range(R))],
                    ins=[bounces[t][:]],
                    outs=[gaths[t][:]],
                )
                h_halves, reload_dmas = load_hT(gaths[t])
                reload_pin = reload_dmas[1]
                # paced DVE-only delay chain anchored on the bounce DMA with
                # a tap gating the next step's x-matmul burst
                dprev = None
                delay_gate = {}
                for i in range(10):
                    dt_ = spool.tile([B, NG], BF16, tag="dly")
                    cp = nc.vector.tensor_copy(
                        dt_[:], wih_t[0][:B, :] if dprev is None else dprev[:]
                    )
                    if i == 0:
                        tile.add_dep_helper(
                            cp.ins,
                            prev_bounce_dma.ins,
                            sync=True,
                            reason="delay chain anchored to bounce",
                        )
                    if i == 9:
                        delay_gate["late"] = cp
                    dprev = dt_
            # final output written from the bounce copy, pinned behind the
            # reload so its HBM traffic misses the AllGather span
            od = nc.scalar.dma_start(out[t, :, :], bounces[t][:])
            if reload_pin is not None:
                tile.add_dep_helper(
                    od.ins, reload_pin.ins, sync=True,
                    reason="keep output writeback off the AllGather span",
                )

    nc.compile()
    return nc


def _prep_inputs(sequence, hidden_state, cell_state, W_ih, W_hh, b_ih, b_hh):
    x = np.asarray(sequence, np.float32)[0]          # [64, 1024]
    h0 = np.asarray(hidden_state, np.float32)[0]     # [64, 1536]
    c0f = np.asarray(cell_state, np.float32)[0]
    W_ih = np.asarray(W_ih, np.float32)
    W_hh = np.asarray(W_hh, np.float32)
    b = (np.asarray(b_ih, np.float32) + np.asarray(b_hh, np.float32))

    bf = ml_dtypes.bfloat16
    xT = np.zeros((KA, B), np.float32)
    xT[:IN] = x.T
    xT[IN] = 1.0
    xT = xT.astype(bf)
    h0T = np.ascontiguousarray(h0.T).astype(bf)
    iden = np.eye(B, dtype=bf)

    in_maps = []
    for r in range(R):
        sl = np.arange(r * HL, (r + 1) * HL)
        # per-core gate column order: g, f, i, o
        sel = np.concatenate([2 * H + sl, H + sl, sl, 3 * H + sl])
        wa = np.zeros((KA, NG), np.float32)
        wa[:IN] = W_ih[sel].T
        wa[IN] = b[sel]
        in_maps.append(
            {
                "xT": xT,
                "wih": wa.astype(bf),
                "whh": np.ascontiguousarray(W_hh[sel].T).astype(bf),
                "h0T": h0T,
                "c0": np.ascontiguousarray(c0f[:, sl]),
                "iden": iden,
            }
        )
    return in_maps


def kernel(**inputs) -> np.ndarray:
    if "nc" not in _CACHE:
        _CACHE["nc"] = _build()
    nc = _CACHE["nc"]
    in_maps = _prep_inputs(**inputs)
    res = bass_utils.run_bass_kernel_spmd(nc, in_maps, core_ids=list(range(R)))
    preds = np.empty((S, B, H), np.float32)
    for r in range(R):
        o = np.asarray(res.results[r]["out"], np.float32)  # [32, 192, 64]
        preds[:, :, r * HL : (r + 1) * HL] = np.transpose(o, (0, 2, 1))
    return preds
